# revision 16
# baseline (speedup 1.0000x reference)
"""Trainium2 Bass kernel for nn_MultiHeadAttention_60851096649901.

Sharding: 8 cores = 4 batches x 2 head-groups (8 heads each).
Each core computes its batch's attention for its 8 heads plus the full
out-projection partial for its head group; host sums the two head-group
partials and adds bo.

Per-core structure (v2 — pipelined emission, warm PE):
  qT/kT = (Wg.T @ x.T + b)           [128, 2048] f32r per head-pair
  v_aug = x @ Wv_aug + bv_aug        [2048, 520] bf16 (65 cols/head, 65th = 1)
  attention per pair, per q-chunk of 512, per k-tile of 128:
    scores for both heads via concurrent row-group matmuls -> [128, 1024] PSUM
    p = exp(8*s - 100) in one [128,1024] ACT instr -> et (f32r SBUF)
    pv[65, 512] += v_aug.T @ p  (PSUM accumulate over 16 k-tiles)
  emission is software-pipelined: scores(kt), exp(kt), fillers, pv(kt-1) --
  so the PV matmul (which waits on exp) never head-of-line-blocks the next
  scores matmul on the PE queue, and the exp stream runs back-to-back.
  normalization (per q-chunk, scheduled into the NEXT window so nothing
  stalls): evict pv, gather both heads' denominators into [2,512], one
  reciprocal_approx_fast, PE outer-product broadcast, DVE multiply.
  out-projection: per q-tile, 8 matmuls accumulate all 4 pairs into one
  PSUM tile (single y output, 8 MB not 32 MB); q-tiles 0-11 run as fillers
  during pair-3 attention, 12-15 in the tail.
PSUM banks: scores ring 2x[128,1024] (4) + pvA/pvB (2) + filler ring (2).
"""

import numpy as np

S = 2048
E = 1024
D = 64
P = 128
HCORE = 8          # heads per core
NPAIR = 4          # head-pairs per core
C_OFF = 100.0      # softmax constant offset (exp(8*s - C))
INV_SCALE = 8.0    # sqrt(head_dim)

_BUILT = None


def _build():
    import concourse.bass as bass
    import concourse.tile as tile
    from concourse import bacc, mybir

    f32 = mybir.dt.float32
    f32r = mybir.dt.float32r
    bf16 = mybir.dt.bfloat16
    Exp = mybir.ActivationFunctionType.Exp

    nc = bacc.Bacc("TRN2", target_bir_lowering=False, debug=False, num_devices=8)

    # host pre-packs the [E, *] matrices as [128 partition, 8 i-tile, *] so
    # each load is ONE dma_start (sync-engine issue cost ~650ns each)
    xT_d = nc.dram_tensor("xT", [P, 8, S], f32, kind="ExternalInput")
    wq_d = nc.dram_tensor("wq", [P, 8, 512], f32, kind="ExternalInput")
    wk_d = nc.dram_tensor("wk", [P, 8, 512], f32, kind="ExternalInput")
    bq_d = nc.dram_tensor("bq", [4, 1, P], f32, kind="ExternalInput")
    bk_d = nc.dram_tensor("bk", [4, 1, P], f32, kind="ExternalInput")
    wv_d = nc.dram_tensor("wv", [P, 8, 520], f32, kind="ExternalInput")
    bv_d = nc.dram_tensor("bv", [1, 1032], f32, kind="ExternalInput")
    wo_d = nc.dram_tensor("wo", [512, E], bf16, kind="ExternalInput")
    y_d = nc.dram_tensor("y", [S, E], f32, kind="ExternalOutput")

    with tile.TileContext(nc) as tc:
        with (
            tc.tile_pool(name="persist", bufs=1) as persist,
            tc.tile_pool(name="wpool", bufs=2) as wpool,
            tc.tile_pool(name="qk", bufs=2) as qkpool,
            tc.tile_pool(name="et", bufs=2) as etpool,
            tc.tile_pool(name="pvc", bufs=2) as pvcpool,
            tc.tile_pool(name="dn", bufs=1) as dnpool,
            tc.tile_pool(name="rcp", bufs=1) as rcpool,
            tc.tile_pool(name="ysb", bufs=2) as ysbpool,
            tc.tile_pool(name="sc", bufs=2, space="PSUM") as scps,     # scores only
            tc.tile_pool(name="pv", bufs=1, space="PSUM") as pvps,     # pv A/B
            tc.tile_pool(name="fill", bufs=2, space="PSUM") as fillps,  # everything else
        ):
            # ---- persistent tiles -------------------------------------------
            xT = persist.tile([P, 8, S], f32r, tag="xT")  # [i-part, i-tile, tok]
            v_sb = persist.tile([P, 16, 520], bf16, tag="v_sb")
            wv = persist.tile([P, 8, 520], f32r, tag="wv")

            neg_c = persist.tile([P, 1], f32, tag="neg_c")
            nc.vector.memset(neg_c[:], -C_OFF)

            bv_r = persist.tile([1, 1032], f32r, tag="bv_r")
            nc.sync.dma_start(bv_r[:], bv_d[:].bitcast(f32r))
            ones_r = bv_r[:, 520:1032]  # host packs ones after bv_aug

            # all 4 pairs' Wo slices and outhT persist until the out-projection
            wo_t = [persist.tile([P, E], bf16, tag=f"wo{j}", name=f"wo{j}")
                    for j in range(NPAIR)]
            outh_t = [persist.tile([P, S], bf16, tag=f"oh{j}", name=f"oh{j}")
                      for j in range(NPAIR)]

            def dma_xT_chunk(cc):
                cs = slice(cc * 512, (cc + 1) * 512)
                nc.sync.dma_start(xT[:, :, cs], xT_d[:, :, cs].bitcast(f32r))

            # ---- op generators (each closure ~1-2 matmuls of PE work) -------
            def v_chunk_ops(kt, c):
                """v_aug[:, kt, chunk c] = x @ Wv_aug + bv (5 closures)."""
                st = {}
                cs = slice(c * 260, (c + 1) * 260)
                ops = []

                def mk(i0):
                    def op():
                        if i0 == 0:
                            st["p"] = fillps.tile([P, 260], f32, tag="fill",
                                                  name=f"vps{kt}_{c}")
                        for i in (i0, i0 + 1):
                            nc.tensor.matmul(
                                st["p"][:], xT[:, i, kt * P:(kt + 1) * P],
                                wv[:, i, cs], start=(i == 0), stop=False,
                            )
                    return op

                for i0 in range(0, 8, 2):
                    ops.append(mk(i0))

                def fin():
                    nc.tensor.matmul(
                        st["p"][:], ones_r[:, 0:P], bv_r[:, cs],
                        start=False, stop=True,
                    )
                    nc.vector.tensor_copy(v_sb[:, kt, cs], st["p"][:])
                ops.append(fin)
                return ops

            def proj_chunk_ops(w, br, dst, c4, nm):
                """qT/kT 512-token chunk c4: 5 closures (8 MM + bias + evict)."""
                st = {}
                ops = []

                def mk(i0):
                    def op():
                        if i0 == 0:
                            st["p"] = fillps.tile([P, 512], f32, tag="fill",
                                                  name=f"pp{nm}_{c4}")
                        for i in (i0, i0 + 1):
                            nc.tensor.matmul(
                                st["p"][:], w[:, i, :],
                                xT[:, i, c4 * 512:(c4 + 1) * 512],
                                start=(i == 0), stop=False,
                            )
                    return op

                for i0 in range(0, 8, 2):
                    ops.append(mk(i0))

                def fin():
                    nc.tensor.matmul(
                        st["p"][:], br, ones_r[:, 0:512],
                        start=False, stop=True,
                    )
                    nc.vector.tensor_copy(dst[:, c4 * 512:(c4 + 1) * 512],
                                          st["p"][:])
                ops.append(fin)
                return ops

            def load_pair_weights(jt):
                js = slice(jt * P, (jt + 1) * P)
                wq = wpool.tile([P, 8, P], f32r, tag="wq", name=f"wq{jt}")
                wk = wpool.tile([P, 8, P], f32r, tag="wk", name=f"wk{jt}")
                nc.sync.dma_start(wq[:], wq_d[:, :, js].bitcast(f32r))
                nc.sync.dma_start(wk[:], wk_d[:, :, js].bitcast(f32r))
                bqr = wpool.tile([1, P], f32r, tag="bqr", name=f"bqr{jt}")
                bkr = wpool.tile([1, P], f32r, tag="bkr", name=f"bkr{jt}")
                nc.sync.dma_start(bqr[:], bq_d[jt].bitcast(f32r))
                nc.sync.dma_start(bkr[:], bk_d[jt].bitcast(f32r))
                nc.sync.dma_start(wo_t[jt][:], wo_d[js, :])
                return wq, wk, bqr, bkr

            def proj_pair_ops(jt, wq, wk, bqr, bkr):
                """Interleave K/Q chunks in consumption order (K first)."""
                qT = qkpool.tile([P, S], f32r, tag="qT", name=f"qT{jt}")
                kT = qkpool.tile([P, S], f32r, tag="kT", name=f"kT{jt}")
                ops = []
                for c4 in range(4):
                    ops.extend(proj_chunk_ops(wk, bkr, kT, c4, f"k{jt}"))
                for c4 in range(4):
                    ops.extend(proj_chunk_ops(wq, bqr, qT, c4, f"q{jt}"))
                return qT, kT, ops

            def outproj_qt_ops(qt):
                """One q-tile of y: accumulate all 4 pairs in PSUM, 1 output."""
                st = {}
                ops = []

                def mk(e, jh):
                    def op():
                        if jh == 0:
                            st[e] = fillps.tile([P, 512], f32, tag="fill",
                                                name=f"yp{qt}_{e}")
                        for j in (jh * 2, jh * 2 + 1):
                            nc.tensor.matmul(
                                st[e][:],
                                outh_t[j][:, qt * P:(qt + 1) * P],
                                wo_t[j][:, e * 512:(e + 1) * 512],
                                start=(j == 0), stop=(j == 3),
                            )
                    return op

                def fin():
                    yb = ysbpool.tile([P, 1024], f32, tag="ysb",
                                      name=f"ysb{qt}")
                    nc.vector.tensor_copy(yb[:, 0:512], st[0][:])
                    nc.vector.tensor_copy(yb[:, 512:1024], st[1][:])
                    nc.sync.dma_start(y_d[qt * P:(qt + 1) * P, :], yb[:])

                ops = [mk(0, 0), mk(0, 1), mk(1, 0), mk(1, 1), fin]
                return ops

            # ---- normalization stages (pipelined into the next window) ------
            def norm_stage1(jt, qc, pvA, pvB):
                """Evict pv (frees PSUM), gather denominators. DVE only."""
                pvca = pvcpool.tile([65, 512], f32, tag="pvc",
                                    name=f"pvc{jt}_{qc}_0")
                pvcb = pvcpool.tile([65, 512], f32, tag="pvc2",
                                    name=f"pvc{jt}_{qc}_1")
                nc.vector.tensor_copy(pvca[:], pvA[:])
                nc.vector.tensor_copy(pvcb[:], pvB[:])
                dnt = dnpool.tile([1, 1024], f32, tag="dn", name=f"dn{jt}_{qc}")
                nc.vector.tensor_copy(dnt[0:1, 0:512], pvca[64:65, :])
                nc.vector.tensor_copy(dnt[0:1, 512:1024], pvcb[64:65, :])
                return {"jt": jt, "qc": qc, "pvc": (pvca, pvcb), "dn": dnt}

            def norm_recip(ns):
                rcf = rcpool.tile([1, 1024], f32, tag="rcf",
                                  name=f"rcf{ns['jt']}_{ns['qc']}")
                rct = rcpool.tile([1, 1024], f32r, tag="rc",
                                  name=f"rc{ns['jt']}_{ns['qc']}")
                nc.vector.reciprocal_approx_fast(out=rcf[:], in_=ns["dn"][:])
                with nc.allow_low_precision(
                        reason="softmax 1/sum in f32r is plenty"):
                    nc.vector.tensor_copy(rct[:], rcf[:])
                ns["rc"] = rct

            def norm_mul(ns, h2):
                jt, qc = ns["jt"], ns["qc"]
                bc = fillps.tile([64, 512], f32, tag="fill",
                                 name=f"bc{jt}_{qc}_{h2}")
                nc.tensor.matmul(bc[:], ones_r[:, 0:64],
                                 ns["rc"][0:1, h2 * 512:(h2 + 1) * 512],
                                 start=True, stop=True)
                nc.vector.tensor_mul(
                    outh_t[jt][h2 * 64:h2 * 64 + 64, qc * 512:(qc + 1) * 512],
                    ns["pvc"][h2][0:64, :], bc[:])

            # ---- upfront: pair-0 weights, K + Q0 proj, full V c=0 ------------
            # DMA emission order = first-need order: xT chunk 0, pair-0
            # weights, wv (V proj), remaining xT chunks
            dma_xT_chunk(0)
            pw = {0: load_pair_weights(0)}
            nc.sync.dma_start(wv[:], wv_d[:].bitcast(f32r))
            for cc in range(1, 4):
                dma_xT_chunk(cc)
            qk = {}
            qT0 = qkpool.tile([P, S], f32r, tag="qT", name="qT0")
            kT0 = qkpool.tile([P, S], f32r, tag="kT", name="kT0")
            qk[0] = (qT0, kT0)
            for cc in range(4):
                for op in proj_chunk_ops(pw[0][1], pw[0][3], kT0, cc, "k0"):
                    op()
                if cc == 0:
                    for op in proj_chunk_ops(pw[0][0], pw[0][2], qT0, 0, "q0"):
                        op()
                for kt in range(cc * 4, cc * 4 + 4):
                    for op in v_chunk_ops(kt, 0):
                        op()

            # ---- attention: 4 pairs x 4 q-chunks x 16 k-tiles ----------------
            pending = None       # normalization state carried into next window
            for jt in range(NPAIR):
                qT, kT = qk[jt]

                # opportunistic filler queue for this pair
                fillers = []
                if jt == 0:
                    # remaining Q-proj chunks for pair 0 (needed at qc 1,2,3)
                    for c4 in range(1, 4):
                        fillers.extend(
                            proj_chunk_ops(pw[0][0], pw[0][2], qT0, c4, "q0"))
                    # second-half V columns, first 6 k-tiles
                    for kt in range(6):
                        fillers.extend(v_chunk_ops(kt, 1))
                if jt in (0, 1, 2):
                    pw[jt + 1] = load_pair_weights(jt + 1)
                    qTn, kTn, opsn = proj_pair_ops(jt + 1, *pw[jt + 1])
                    qk[jt + 1] = (qTn, kTn)
                    fillers.extend(opsn)
                if jt == 1:
                    for kt in range(6, 16):
                        fillers.extend(v_chunk_ops(kt, 1))
                fillers.reverse()  # pop() from the front, in order

                n_iters = 64
                it = 0
                for qc in range(4):
                    qs = slice(qc * 512, (qc + 1) * 512)
                    pvA = pvps.tile([65, 512], f32, tag="pvA",
                                    name=f"pvA{jt}_{qc}")
                    pvB = pvps.tile([65, 512], f32, tag="pvB",
                                    name=f"pvB{jt}_{qc}")
                    opq = []
                    if jt == 3 and qc >= 1:
                        for qt in range((qc - 1) * 4, (qc - 1) * 4 + 4):
                            opq.extend(outproj_qt_ops(qt))
                    prev_pv = None
                    for kt in range(16):
                        # scores for both heads (concurrent row-group matmuls)
                        sct = scps.tile([P, 1024], f32, tag="sc")
                        for h2 in range(2):
                            hb = h2 * 64
                            nc.tensor.matmul(
                                sct[:, h2 * 512:(h2 + 1) * 512],
                                kT[hb:hb + 64, kt * P:(kt + 1) * P],
                                qT[hb:hb + 64, qs],
                                start=True, stop=True,
                            )
                        et = etpool.tile([P, 1024], bf16, tag="exp")
                        nc.scalar.activation(
                            out=et[:], in_=sct[:], func=Exp,
                            bias=neg_c[:], scale=INV_SCALE,
                        )

                        # scheduled work for this slot (never blocks the
                        # scores/exp stream)
                        if kt == 2 and pending is not None:
                            norm_recip(pending)
                        elif kt == 4 and pending is not None:
                            norm_mul(pending, 0)
                        elif kt == 6 and pending is not None:
                            norm_mul(pending, 1)
                            pending = None
                        elif kt >= 7 and opq:
                            opq.pop(0)()
                            if opq:
                                opq.pop(0)()

                        # opportunistic fillers
                        it += 1
                        remaining = n_iters - it
                        budget = 2 if len(fillers) > remaining else (
                            1 if fillers else 0)
                        for _ in range(budget):
                            if fillers:
                                fillers.pop()()

                        # previous iteration's PV (after this iter's scores so
                        # it can't head-of-line-block them while waiting on exp)
                        if prev_pv is not None:
                            prev_pv()

                        def mk_pv(et_=et, kt_=kt):
                            def op():
                                for h2, pv in ((0, pvA), (1, pvB)):
                                    h = jt * 2 + h2
                                    nc.tensor.matmul(
                                        pv[:],
                                        v_sb[:, kt_, h * 65:h * 65 + 65],
                                        et_[:, h2 * 512:(h2 + 1) * 512],
                                        start=(kt_ == 0), stop=(kt_ == 15),
                                    )
                            return op
                        prev_pv = mk_pv()
                    prev_pv()
                    while opq:
                        opq.pop(0)()
                    while fillers and qc == 3 and jt < 3:
                        fillers.pop()()  # safety drain (should be empty)
                    pending_new = norm_stage1(jt, qc, pvA, pvB)
                    if pending is not None:
                        # should not happen (slots above consume it), but keep
                        # the chain correct if schedule shifts
                        norm_recip(pending)
                        norm_mul(pending, 0)
                        norm_mul(pending, 1)
                    pending = pending_new

            # ---- tail: last normalization + final out-projection -------------
            norm_recip(pending)
            norm_mul(pending, 0)
            norm_mul(pending, 1)
            for qt in range(12, 16):
                for op in outproj_qt_ops(qt):
                    op()

    nc.compile()
    return nc


def _get_nc():
    global _BUILT
    if _BUILT is None:
        _BUILT = _build()
    return _BUILT


def _itile_pack(m):
    """[E, C] -> [128, 8, C]: partition-major i-tile layout for 1-DMA loads."""
    return np.ascontiguousarray(
        m.reshape(8, P, m.shape[1]).transpose(1, 0, 2))


def _prep_core_inputs(x, Wq, bq, Wk, bk, Wv, bv, Wo, g, b):
    gs = g * 512
    xT = _itile_pack(x[b].T.astype(np.float32))
    wq = _itile_pack(np.ascontiguousarray(Wq[:, gs:gs + 512].astype(np.float32)))
    wk = _itile_pack(np.ascontiguousarray(Wk[:, gs:gs + 512].astype(np.float32)))
    bqs = np.ascontiguousarray(bq[gs:gs + 512].astype(np.float32).reshape(4, 1, P))
    bks = np.ascontiguousarray(bk[gs:gs + 512].astype(np.float32).reshape(4, 1, P))
    wv = np.zeros((E, 520), np.float32)
    bva = np.zeros((1, 1032), np.float32)
    bva[0, 520:] = 1.0
    for h in range(HCORE):
        wv[:, h * 65:h * 65 + 64] = Wv[:, gs + h * 64:gs + (h + 1) * 64]
        bva[0, h * 65:h * 65 + 64] = bv[gs + h * 64:gs + (h + 1) * 64]
        bva[0, h * 65 + 64] = 1.0
    wv = _itile_pack(wv)
    wo = np.ascontiguousarray(Wo[gs:gs + 512, :].astype('bfloat16'))
    return {
        "xT": xT, "wq": wq, "wk": wk, "bq": bqs, "bk": bks,
        "wv": wv, "bv": bva, "wo": wo,
    }


def kernel(x, Wq, bq, Wk, bk, Wv, bv, Wo, bo):
    from concourse.bass_utils import run_bass_kernel_spmd

    x = np.asarray(x)
    B = x.shape[0]
    nc = _get_nc()
    in_maps = []
    for c in range(8):
        g, b = c // 4, c % 4
        in_maps.append(
            _prep_core_inputs(x, np.asarray(Wq), np.asarray(bq), np.asarray(Wk),
                              np.asarray(bk), np.asarray(Wv), np.asarray(bv),
                              np.asarray(Wo), g, b)
        )
    res = run_bass_kernel_spmd(nc, in_maps, list(range(8)))
    y = np.zeros((B, S, E), np.float32)
    bo = np.asarray(bo, dtype=np.float32)
    for c in range(8):
        b = c % 4
        y[b] += res.results[c]["y"]
    y += bo
    return y


# revision 23
# speedup vs baseline: 1.2073x; 1.2073x over previous
"""Trainium2 Bass kernel for nn_MultiHeadAttention_60851096649901.

Sharding: 8 cores = 4 batches x 2 head-groups (8 heads each).
Each core computes its batch's attention for its 8 heads plus the full
out-projection partial for its head group; host sums the two head-group
partials and adds bo.

Per-core structure (v2 — pipelined emission, warm PE):
  qT/kT = (Wg.T @ x.T + b)           [128, 2048] f32r per head-pair
  v_aug = x @ Wv_aug + bv_aug        [2048, 520] bf16 (65 cols/head, 65th = 1)
  attention per pair, per q-chunk of 512, per k-tile of 128:
    scores for both heads via concurrent row-group matmuls -> [128, 1024] PSUM
    p = exp(8*s - 100) in one [128,1024] ACT instr -> et (f32r SBUF)
    pv[65, 512] += v_aug.T @ p  (PSUM accumulate over 16 k-tiles)
  emission is software-pipelined: scores(kt), exp(kt), fillers, pv(kt-1) --
  so the PV matmul (which waits on exp) never head-of-line-blocks the next
  scores matmul on the PE queue, and the exp stream runs back-to-back.
  normalization (per q-chunk, scheduled into the NEXT window so nothing
  stalls): evict pv, gather both heads' denominators into [2,512], one
  reciprocal_approx_fast, PE outer-product broadcast, DVE multiply.
  out-projection: per q-tile, 8 matmuls accumulate all 4 pairs into one
  PSUM tile (single y output, 8 MB not 32 MB); q-tiles 0-11 run as fillers
  during pair-3 attention, 12-15 in the tail.
PSUM banks: scores ring 2x[128,1024] (4) + pvA/pvB (2) + filler ring (2).
"""

import numpy as np

S = 2048
E = 1024
D = 64
P = 128
HCORE = 8          # heads per core
NPAIR = 4          # head-pairs per core
C_OFF = 100.0      # softmax constant offset (exp(8*s - C))
INV_SCALE = 8.0    # sqrt(head_dim)

_BUILT = None


def _build():
    import concourse.bass as bass
    import concourse.tile as tile
    from concourse import bacc, mybir

    f32 = mybir.dt.float32
    f32r = mybir.dt.float32r
    bf16 = mybir.dt.bfloat16
    Exp = mybir.ActivationFunctionType.Exp

    nc = bacc.Bacc("TRN2", target_bir_lowering=False, debug=False, num_devices=8)

    # host pre-packs weights/activations so every DMA is contiguous per
    # partition (descriptor-gen cost scales with segment count): xT is
    # token-chunk-major, wq/wk are pair-major
    xT_d = nc.dram_tensor("xT", [P, 4, 8, 512], f32, kind="ExternalInput")
    wq_d = nc.dram_tensor("wq", [P, 4, 8, P], f32, kind="ExternalInput")
    wk_d = nc.dram_tensor("wk", [P, 4, 8, P], f32, kind="ExternalInput")
    bq_d = nc.dram_tensor("bq", [4, 1, P], f32, kind="ExternalInput")
    bk_d = nc.dram_tensor("bk", [4, 1, P], f32, kind="ExternalInput")
    wv_d = nc.dram_tensor("wv", [P, 8, 520], f32, kind="ExternalInput")
    bv_d = nc.dram_tensor("bv", [1, 1032], f32, kind="ExternalInput")
    wo_d = nc.dram_tensor("wo", [512, E], bf16, kind="ExternalInput")
    y_d = nc.dram_tensor("y", [S, E], f32, kind="ExternalOutput")

    with tile.TileContext(nc) as tc:
        with (
            tc.tile_pool(name="persist", bufs=1) as persist,
            tc.tile_pool(name="wpool", bufs=2) as wpool,
            tc.tile_pool(name="qk", bufs=2) as qkpool,
            tc.tile_pool(name="et", bufs=2) as etpool,
            tc.tile_pool(name="pvc", bufs=2) as pvcpool,
            tc.tile_pool(name="dn", bufs=1) as dnpool,
            tc.tile_pool(name="rcp", bufs=1) as rcpool,
            tc.tile_pool(name="ysb", bufs=2) as ysbpool,
            tc.tile_pool(name="sc", bufs=2, space="PSUM") as scps,     # scores only
            tc.tile_pool(name="pv", bufs=1, space="PSUM") as pvps,     # pv A/B
            tc.tile_pool(name="fill", bufs=2, space="PSUM") as fillps,  # everything else
        ):
            # ---- persistent tiles -------------------------------------------
            # xT layout: [i-part, token-chunk, i-tile, token-within-chunk]
            xT = persist.tile([P, 4, 8, 512], f32r, tag="xT")

            def xt_proj(i, c4):
                """rhs slice for projections: i-tile x 512-token chunk."""
                return xT[:, c4, i, :]

            def xt_ktile(i, kt):
                """lhsT slice for the V projection: i-tile x 128-token tile."""
                o = (kt % 4) * P
                return xT[:, kt // 4, i, o:o + P]
            v_sb = persist.tile([P, 16, 520], bf16, tag="v_sb")
            wv = persist.tile([P, 8, 520], f32r, tag="wv")

            neg_c = persist.tile([P, 1], f32, tag="neg_c")
            nc.vector.memset(neg_c[:], -C_OFF)

            bv_r = persist.tile([1, 1032], f32r, tag="bv_r")
            nc.sync.dma_start(bv_r[:], bv_d[:].bitcast(f32r))
            ones_r = bv_r[:, 520:1032]  # host packs ones after bv_aug

            # all 4 pairs' Wo slices and outhT persist until the out-projection
            wo_t = [persist.tile([P, E], bf16, tag=f"wo{j}", name=f"wo{j}")
                    for j in range(NPAIR)]
            outh_t = [persist.tile([P, S], bf16, tag=f"oh{j}", name=f"oh{j}")
                      for j in range(NPAIR)]

            def dma_xT_chunk(cc):
                nc.sync.dma_start(xT[:, cc], xT_d[:, cc].bitcast(f32r))

            # ---- op generators (each closure ~1-2 matmuls of PE work) -------
            def v_chunk_ops(kt, c):
                """v_aug[:, kt, chunk c] = x @ Wv_aug + bv (5 closures)."""
                st = {}
                cs = slice(c * 260, (c + 1) * 260)
                ops = []

                def mk(i0):
                    def op():
                        if i0 == 0:
                            st["p"] = fillps.tile([P, 260], f32, tag="fill",
                                                  name=f"vps{kt}_{c}")
                        for i in (i0, i0 + 1):
                            nc.tensor.matmul(
                                st["p"][:], xt_ktile(i, kt),
                                wv[:, i, cs], start=(i == 0), stop=False,
                            )
                    return op

                for i0 in range(0, 8, 2):
                    ops.append(mk(i0))

                def fin():
                    nc.tensor.matmul(
                        st["p"][:], ones_r[:, 0:P], bv_r[:, cs],
                        start=False, stop=True,
                    )
                    nc.vector.tensor_copy(v_sb[:, kt, cs], st["p"][:])
                ops.append(fin)
                return ops

            def proj_chunk_ops(w, br, dst, c4, nm):
                """qT/kT 512-token chunk c4: 5 closures (8 MM + bias + evict)."""
                st = {}
                ops = []

                def mk(i0):
                    def op():
                        if i0 == 0:
                            st["p"] = fillps.tile([P, 512], f32, tag="fill",
                                                  name=f"pp{nm}_{c4}")
                        for i in (i0, i0 + 1):
                            nc.tensor.matmul(
                                st["p"][:], w[:, i, :], xt_proj(i, c4),
                                start=(i == 0), stop=False,
                            )
                    return op

                for i0 in range(0, 8, 2):
                    ops.append(mk(i0))

                def fin():
                    nc.tensor.matmul(
                        st["p"][:], br, ones_r[:, 0:512],
                        start=False, stop=True,
                    )
                    nc.vector.tensor_copy(dst[:, c4 * 512:(c4 + 1) * 512],
                                          st["p"][:])
                ops.append(fin)
                return ops

            def load_pair_weights(jt):
                js = slice(jt * P, (jt + 1) * P)
                wq = wpool.tile([P, 8, P], f32r, tag="wq", name=f"wq{jt}")
                wk = wpool.tile([P, 8, P], f32r, tag="wk", name=f"wk{jt}")
                nc.sync.dma_start(wq[:], wq_d[:, jt].bitcast(f32r))
                nc.sync.dma_start(wk[:], wk_d[:, jt].bitcast(f32r))
                bqr = wpool.tile([1, P], f32r, tag="bqr", name=f"bqr{jt}")
                bkr = wpool.tile([1, P], f32r, tag="bkr", name=f"bkr{jt}")
                nc.sync.dma_start(bqr[:], bq_d[jt].bitcast(f32r))
                nc.sync.dma_start(bkr[:], bk_d[jt].bitcast(f32r))
                nc.sync.dma_start(wo_t[jt][:], wo_d[js, :])
                return wq, wk, bqr, bkr

            def proj_pair_ops(jt, wq, wk, bqr, bkr):
                """Interleave K/Q chunks in consumption order (K first)."""
                qT = qkpool.tile([P, S], f32r, tag="qT", name=f"qT{jt}")
                kT = qkpool.tile([P, S], f32r, tag="kT", name=f"kT{jt}")
                ops = []
                for c4 in range(4):
                    ops.extend(proj_chunk_ops(wk, bkr, kT, c4, f"k{jt}"))
                for c4 in range(4):
                    ops.extend(proj_chunk_ops(wq, bqr, qT, c4, f"q{jt}"))
                return qT, kT, ops

            def outproj_qt_ops(qt):
                """One q-tile of y: accumulate all 4 pairs in PSUM, 1 output."""
                st = {}
                ops = []

                def mk(e, jh):
                    def op():
                        if jh == 0:
                            st[e] = fillps.tile([P, 512], f32, tag="fill",
                                                name=f"yp{qt}_{e}")
                        for j in (jh * 2, jh * 2 + 1):
                            nc.tensor.matmul(
                                st[e][:],
                                outh_t[j][:, qt * P:(qt + 1) * P],
                                wo_t[j][:, e * 512:(e + 1) * 512],
                                start=(j == 0), stop=(j == 3),
                            )
                    return op

                def fin():
                    yb = ysbpool.tile([P, 1024], f32, tag="ysb",
                                      name=f"ysb{qt}")
                    nc.vector.tensor_copy(yb[:, 0:512], st[0][:])
                    nc.vector.tensor_copy(yb[:, 512:1024], st[1][:])
                    nc.sync.dma_start(y_d[qt * P:(qt + 1) * P, :], yb[:])

                ops = [mk(0, 0), mk(0, 1), mk(1, 0), mk(1, 1), fin]
                return ops

            # ---- normalization stages (pipelined into the next window) ------
            def norm_stage1(jt, qc, pvA, pvB):
                """Evict pv (frees PSUM), gather denominators. DVE only."""
                pvca = pvcpool.tile([65, 512], f32, tag="pvc",
                                    name=f"pvc{jt}_{qc}_0")
                pvcb = pvcpool.tile([65, 512], f32, tag="pvc2",
                                    name=f"pvc{jt}_{qc}_1")
                nc.vector.tensor_copy(pvca[:], pvA[:])
                nc.vector.tensor_copy(pvcb[:], pvB[:])
                dnt = dnpool.tile([1, 1024], f32, tag="dn", name=f"dn{jt}_{qc}")
                nc.vector.tensor_copy(dnt[0:1, 0:512], pvca[64:65, :])
                nc.vector.tensor_copy(dnt[0:1, 512:1024], pvcb[64:65, :])
                return {"jt": jt, "qc": qc, "pvc": (pvca, pvcb), "dn": dnt}

            def norm_recip(ns):
                rcf = rcpool.tile([1, 1024], f32, tag="rcf",
                                  name=f"rcf{ns['jt']}_{ns['qc']}")
                rct = rcpool.tile([1, 1024], f32r, tag="rc",
                                  name=f"rc{ns['jt']}_{ns['qc']}")
                nc.vector.reciprocal_approx_fast(out=rcf[:], in_=ns["dn"][:])
                with nc.allow_low_precision(
                        reason="softmax 1/sum in f32r is plenty"):
                    nc.vector.tensor_copy(rct[:], rcf[:])
                ns["rc"] = rct

            def norm_mul(ns, h2):
                jt, qc = ns["jt"], ns["qc"]
                bc = fillps.tile([64, 512], f32, tag="fill",
                                 name=f"bc{jt}_{qc}_{h2}")
                nc.tensor.matmul(bc[:], ones_r[:, 0:64],
                                 ns["rc"][0:1, h2 * 512:(h2 + 1) * 512],
                                 start=True, stop=True)
                nc.vector.tensor_mul(
                    outh_t[jt][h2 * 64:h2 * 64 + 64, qc * 512:(qc + 1) * 512],
                    ns["pvc"][h2][0:64, :], bc[:])

            # ---- upfront: pair-0 weights, K + Q0 proj, full V c=0 ------------
            # DMA emission order = first-need order: xT chunk 0, pair-0
            # weights, wv (V proj), remaining xT chunks
            dma_xT_chunk(0)
            pw = {0: load_pair_weights(0)}
            nc.sync.dma_start(wv[:], wv_d[:].bitcast(f32r))
            for cc in range(1, 4):
                dma_xT_chunk(cc)
            qk = {}
            qT0 = qkpool.tile([P, S], f32r, tag="qT", name="qT0")
            kT0 = qkpool.tile([P, S], f32r, tag="kT", name="kT0")
            qk[0] = (qT0, kT0)
            for cc in range(4):
                for op in proj_chunk_ops(pw[0][1], pw[0][3], kT0, cc, "k0"):
                    op()
                if cc == 0:
                    for op in proj_chunk_ops(pw[0][0], pw[0][2], qT0, 0, "q0"):
                        op()
                for kt in range(cc * 4, cc * 4 + 4):
                    for op in v_chunk_ops(kt, 0):
                        op()

            # ---- attention: 4 pairs x 4 q-chunks x 16 k-tiles ----------------
            pending = None       # normalization state carried into next window
            for jt in range(NPAIR):
                qT, kT = qk[jt]

                # opportunistic filler queue for this pair
                fillers = []
                if jt == 0:
                    # remaining Q-proj chunks for pair 0 (needed at qc 1,2,3)
                    for c4 in range(1, 4):
                        fillers.extend(
                            proj_chunk_ops(pw[0][0], pw[0][2], qT0, c4, "q0"))
                    # second-half V columns, first 6 k-tiles
                    for kt in range(6):
                        fillers.extend(v_chunk_ops(kt, 1))
                if jt in (0, 1, 2):
                    pw[jt + 1] = load_pair_weights(jt + 1)
                    qTn, kTn, opsn = proj_pair_ops(jt + 1, *pw[jt + 1])
                    qk[jt + 1] = (qTn, kTn)
                    fillers.extend(opsn)
                if jt == 1:
                    for kt in range(6, 16):
                        fillers.extend(v_chunk_ops(kt, 1))
                fillers.reverse()  # pop() from the front, in order

                n_iters = 64
                it = 0
                for qc in range(4):
                    qs = slice(qc * 512, (qc + 1) * 512)
                    pvA = pvps.tile([65, 512], f32, tag="pvA",
                                    name=f"pvA{jt}_{qc}")
                    pvB = pvps.tile([65, 512], f32, tag="pvB",
                                    name=f"pvB{jt}_{qc}")
                    opq = []
                    if jt == 3 and qc >= 1:
                        for qt in range((qc - 1) * 4, (qc - 1) * 4 + 4):
                            opq.extend(outproj_qt_ops(qt))
                    prev_pv = None
                    for kt in range(16):
                        # scores for both heads (concurrent row-group matmuls)
                        sct = scps.tile([P, 1024], f32, tag="sc")
                        for h2 in range(2):
                            hb = h2 * 64
                            nc.tensor.matmul(
                                sct[:, h2 * 512:(h2 + 1) * 512],
                                kT[hb:hb + 64, kt * P:(kt + 1) * P],
                                qT[hb:hb + 64, qs],
                                start=True, stop=True,
                            )
                        et = etpool.tile([P, 1024], bf16, tag="exp")
                        nc.scalar.activation(
                            out=et[:], in_=sct[:], func=Exp,
                            bias=neg_c[:], scale=INV_SCALE,
                        )

                        # scheduled work for this slot (never blocks the
                        # scores/exp stream)
                        if kt == 2 and pending is not None:
                            norm_recip(pending)
                        elif kt == 4 and pending is not None:
                            norm_mul(pending, 0)
                        elif kt == 6 and pending is not None:
                            norm_mul(pending, 1)
                            pending = None
                        elif kt >= 7 and opq:
                            opq.pop(0)()
                            if opq:
                                opq.pop(0)()

                        # opportunistic fillers
                        it += 1
                        remaining = n_iters - it
                        budget = 2 if len(fillers) > remaining else (
                            1 if fillers else 0)
                        for _ in range(budget):
                            if fillers:
                                fillers.pop()()

                        # previous iteration's PV (after this iter's scores so
                        # it can't head-of-line-block them while waiting on exp)
                        if prev_pv is not None:
                            prev_pv()

                        def mk_pv(et_=et, kt_=kt):
                            def op():
                                for h2, pv in ((0, pvA), (1, pvB)):
                                    h = jt * 2 + h2
                                    nc.tensor.matmul(
                                        pv[:],
                                        v_sb[:, kt_, h * 65:h * 65 + 65],
                                        et_[:, h2 * 512:(h2 + 1) * 512],
                                        start=(kt_ == 0), stop=(kt_ == 15),
                                    )
                            return op
                        prev_pv = mk_pv()
                    prev_pv()
                    while opq:
                        opq.pop(0)()
                    while fillers and qc == 3 and jt < 3:
                        fillers.pop()()  # safety drain (should be empty)
                    pending_new = norm_stage1(jt, qc, pvA, pvB)
                    if pending is not None:
                        # should not happen (slots above consume it), but keep
                        # the chain correct if schedule shifts
                        norm_recip(pending)
                        norm_mul(pending, 0)
                        norm_mul(pending, 1)
                    pending = pending_new

            # ---- tail: last normalization + final out-projection -------------
            norm_recip(pending)
            norm_mul(pending, 0)
            norm_mul(pending, 1)
            for qt in range(12, 16):
                for op in outproj_qt_ops(qt):
                    op()

    nc.compile()
    return nc


def _get_nc():
    global _BUILT
    if _BUILT is None:
        _BUILT = _build()
    return _BUILT


def _itile_pack(m):
    """[E, C] -> [128, 8, C]: partition-major i-tile layout for 1-DMA loads."""
    return np.ascontiguousarray(
        m.reshape(8, P, m.shape[1]).transpose(1, 0, 2))


def _xt_pack(m):
    """[E, S] -> [128, 4, 8, 512]: per-partition-contiguous chunk-major."""
    return np.ascontiguousarray(
        m.reshape(8, P, 4, 512).transpose(1, 2, 0, 3))


def _w_pack(m):
    """[E, 512] -> [128, 4, 8, 128]: per-partition-contiguous pair-major."""
    return np.ascontiguousarray(
        m.reshape(8, P, 4, P).transpose(1, 2, 0, 3))


def _prep_core_inputs(x, Wq, bq, Wk, bk, Wv, bv, Wo, g, b):
    gs = g * 512
    xT = _xt_pack(x[b].T.astype(np.float32))
    wq = _w_pack(np.ascontiguousarray(Wq[:, gs:gs + 512].astype(np.float32)))
    wk = _w_pack(np.ascontiguousarray(Wk[:, gs:gs + 512].astype(np.float32)))
    bqs = np.ascontiguousarray(bq[gs:gs + 512].astype(np.float32).reshape(4, 1, P))
    bks = np.ascontiguousarray(bk[gs:gs + 512].astype(np.float32).reshape(4, 1, P))
    wv = np.zeros((E, 520), np.float32)
    bva = np.zeros((1, 1032), np.float32)
    bva[0, 520:] = 1.0
    for h in range(HCORE):
        wv[:, h * 65:h * 65 + 64] = Wv[:, gs + h * 64:gs + (h + 1) * 64]
        bva[0, h * 65:h * 65 + 64] = bv[gs + h * 64:gs + (h + 1) * 64]
        bva[0, h * 65 + 64] = 1.0
    wv = _itile_pack(wv)
    wo = np.ascontiguousarray(Wo[gs:gs + 512, :].astype('bfloat16'))
    return {
        "xT": xT, "wq": wq, "wk": wk, "bq": bqs, "bk": bks,
        "wv": wv, "bv": bva, "wo": wo,
    }


def kernel(x, Wq, bq, Wk, bk, Wv, bv, Wo, bo):
    from concourse.bass_utils import run_bass_kernel_spmd

    x = np.asarray(x)
    B = x.shape[0]
    nc = _get_nc()
    in_maps = []
    for c in range(8):
        g, b = c // 4, c % 4
        in_maps.append(
            _prep_core_inputs(x, np.asarray(Wq), np.asarray(bq), np.asarray(Wk),
                              np.asarray(bk), np.asarray(Wv), np.asarray(bv),
                              np.asarray(Wo), g, b)
        )
    res = run_bass_kernel_spmd(nc, in_maps, list(range(8)))
    y = np.zeros((B, S, E), np.float32)
    bo = np.asarray(bo, dtype=np.float32)
    for c in range(8):
        b = c % 4
        y[b] += res.results[c]["y"]
    y += bo
    return y


# revision 31
# speedup vs baseline: 1.2983x; 1.0753x over previous
"""Trainium2 Bass kernel for nn_MultiHeadAttention_60851096649901.

Sharding: 8 cores = 4 batches x 2 head-groups (8 heads each).
Each core computes its batch's attention for its 8 heads plus the full
out-projection partial for its head group; host sums the two head-group
partials and adds bo.

Per-core structure (v2 — pipelined emission, warm PE):
  qT/kT = (Wg.T @ x.T + b)           [128, 2048] f32r per head-pair
  v_aug = x @ Wv_aug + bv_aug        [2048, 520] bf16 (65 cols/head, 65th = 1)
  attention per pair, per q-chunk of 512, per k-tile of 128:
    scores for both heads via concurrent row-group matmuls -> [128, 1024] PSUM
    p = exp(8*s - 100) in one [128,1024] ACT instr -> et (f32r SBUF)
    pv[65, 512] += v_aug.T @ p  (PSUM accumulate over 16 k-tiles)
  emission is software-pipelined: scores(kt), exp(kt), fillers, pv(kt-1) --
  so the PV matmul (which waits on exp) never head-of-line-blocks the next
  scores matmul on the PE queue, and the exp stream runs back-to-back.
  normalization (per q-chunk, scheduled into the NEXT window so nothing
  stalls): evict pv, gather both heads' denominators into [2,512], one
  reciprocal_approx_fast, PE outer-product broadcast, DVE multiply.
  out-projection: per q-tile, 8 matmuls accumulate all 4 pairs into one
  PSUM tile (single y output, 8 MB not 32 MB); q-tiles 0-11 run as fillers
  during pair-3 attention, 12-15 in the tail.
PSUM banks: scores ring 2x[128,1024] (4) + pvA/pvB (2) + filler ring (2).
"""

import numpy as np

S = 2048
E = 1024
D = 64
P = 128
HCORE = 8          # heads per core
NPAIR = 4          # head-pairs per core
C_OFF = 100.0      # softmax constant offset (exp(8*s - C))
INV_SCALE = 8.0    # sqrt(head_dim)

_BUILT = None


def _build():
    import concourse.bass as bass
    import concourse.tile as tile
    from concourse import bacc, mybir

    f32 = mybir.dt.float32
    f32r = mybir.dt.float32r
    bf16 = mybir.dt.bfloat16
    Exp = mybir.ActivationFunctionType.Exp

    nc = bacc.Bacc("TRN2", target_bir_lowering=False, debug=False, num_devices=8)

    # host pre-packs weights/activations so every DMA is contiguous per
    # partition (descriptor-gen cost scales with segment count): xT is
    # token-chunk-major, wq/wk are pair-major
    xT_d = nc.dram_tensor("xT", [P, 4, 8, 512], f32, kind="ExternalInput")
    wq_d = nc.dram_tensor("wq", [P, 4, 8, P], f32, kind="ExternalInput")
    wk_d = nc.dram_tensor("wk", [P, 4, 8, P], f32, kind="ExternalInput")
    bq_d = nc.dram_tensor("bq", [4, P, 1], f32, kind="ExternalInput")
    bk_d = nc.dram_tensor("bk", [4, P, 1], f32, kind="ExternalInput")
    wv_d = nc.dram_tensor("wv", [P, 8, 520], f32, kind="ExternalInput")
    bv_d = nc.dram_tensor("bv", [1, 1032], f32, kind="ExternalInput")
    wo_d = nc.dram_tensor("wo", [512, E], bf16, kind="ExternalInput")
    y_d = nc.dram_tensor("y", [S, E], f32, kind="ExternalOutput")

    with tile.TileContext(nc) as tc:
        with (
            tc.tile_pool(name="persist", bufs=1) as persist,
            tc.tile_pool(name="wpool", bufs=2) as wpool,
            tc.tile_pool(name="qk", bufs=2) as qkpool,
            tc.tile_pool(name="et", bufs=2) as etpool,
            tc.tile_pool(name="pvc", bufs=2) as pvcpool,
            tc.tile_pool(name="dn", bufs=1) as dnpool,
            tc.tile_pool(name="rcp", bufs=1) as rcpool,
            tc.tile_pool(name="bcp", bufs=2) as bcpool,
            tc.tile_pool(name="ysb", bufs=2) as ysbpool,
            tc.tile_pool(name="sc", bufs=2, space="PSUM") as scps,     # scores only
            tc.tile_pool(name="pv", bufs=1, space="PSUM") as pvps,     # pv A/B
            tc.tile_pool(name="fill", bufs=2, space="PSUM") as fillps,  # everything else
        ):
            # ---- persistent tiles -------------------------------------------
            # xT layout: [i-part, token-chunk, i-tile, token-within-chunk]
            xT = persist.tile([P, 4, 8, 512], f32r, tag="xT")

            def xt_proj(i, c4):
                """rhs slice for projections: i-tile x 512-token chunk."""
                return xT[:, c4, i, :]

            def xt_ktile(i, kt):
                """lhsT slice for the V projection: i-tile x 128-token tile."""
                o = (kt % 4) * P
                return xT[:, kt // 4, i, o:o + P]
            v_sb = persist.tile([P, 16, 520], bf16, tag="v_sb")
            wv = persist.tile([P, 8, 520], f32r, tag="wv")

            neg_c = persist.tile([P, 1], f32, tag="neg_c")
            nc.vector.memset(neg_c[:], -C_OFF)

            bv_r = persist.tile([1, 1032], f32r, tag="bv_r")
            nc.sync.dma_start(bv_r[:], bv_d[:].bitcast(f32r))
            ones_r = bv_r[:, 520:1032]  # host packs ones after bv_aug

            # all 4 pairs' Wo slices and outhT persist until the out-projection
            wo_t = [persist.tile([P, E], bf16, tag=f"wo{j}", name=f"wo{j}")
                    for j in range(NPAIR)]
            outh_t = [persist.tile([P, S], bf16, tag=f"oh{j}", name=f"oh{j}")
                      for j in range(NPAIR)]

            def dma_xT_chunk(cc):
                nc.sync.dma_start(xT[:, cc], xT_d[:, cc].bitcast(f32r))

            # ---- op generators (each closure ~1-2 matmuls of PE work) -------
            def v_chunk_ops(kt, c):
                """v_aug[:, kt, chunk c] = x @ Wv_aug + bv (5 closures)."""
                st = {}
                cs = slice(c * 260, (c + 1) * 260)
                ops = []

                def mk(i0):
                    def op():
                        if i0 == 0:
                            st["p"] = fillps.tile([P, 260], f32, tag="fill",
                                                  name=f"vps{kt}_{c}")
                        for i in (i0, i0 + 1):
                            nc.tensor.matmul(
                                st["p"][:], xt_ktile(i, kt),
                                wv[:, i, cs], start=(i == 0), stop=False,
                            )
                    return op

                for i0 in range(0, 8, 2):
                    ops.append(mk(i0))

                def fin():
                    nc.tensor.matmul(
                        st["p"][:], ones_r[:, 0:P], bv_r[:, cs],
                        start=False, stop=True,
                    )
                    nc.vector.tensor_copy(v_sb[:, kt, cs], st["p"][:])
                ops.append(fin)
                return ops

            def proj_chunk_ops(w, br, dst, c4, nm):
                """qT/kT 512-token chunk c4: 5 closures (8 MM + bias + evict)."""
                st = {}
                ops = []

                def mk(i0):
                    def op():
                        if i0 == 0:
                            st["p"] = fillps.tile([P, 512], f32, tag="fill",
                                                  name=f"pp{nm}_{c4}")
                        for i in (i0, i0 + 1):
                            nc.tensor.matmul(
                                st["p"][:], w[:, i, :], xt_proj(i, c4),
                                start=(i == 0), stop=False,
                            )
                    return op

                for i0 in range(0, 8, 2):
                    ops.append(mk(i0))

                def fin():
                    # bias is per-partition (per output dim) -> fold into the
                    # DVE evict instead of a PE matmul
                    nc.vector.tensor_scalar_add(
                        dst[:, c4 * 512:(c4 + 1) * 512], st["p"][:], br[:])
                ops.append(fin)
                return ops

            def load_pair_weights(jt):
                js = slice(jt * P, (jt + 1) * P)
                wq = wpool.tile([P, 8, P], f32r, tag="wq", name=f"wq{jt}")
                wk = wpool.tile([P, 8, P], f32r, tag="wk", name=f"wk{jt}")
                nc.sync.dma_start(wq[:], wq_d[:, jt].bitcast(f32r))
                nc.sync.dma_start(wk[:], wk_d[:, jt].bitcast(f32r))
                bqr = wpool.tile([P, 1], f32, tag="bqr", name=f"bqr{jt}")
                bkr = wpool.tile([P, 1], f32, tag="bkr", name=f"bkr{jt}")
                nc.sync.dma_start(bqr[:], bq_d[jt])
                nc.sync.dma_start(bkr[:], bk_d[jt])
                nc.sync.dma_start(wo_t[jt][:], wo_d[js, :])
                return wq, wk, bqr, bkr

            def proj_pair_ops(jt, wq, wk, bqr, bkr):
                """Interleave K/Q chunks in consumption order (K first)."""
                qT = qkpool.tile([P, S], f32r, tag="qT", name=f"qT{jt}")
                kT = qkpool.tile([P, S], f32r, tag="kT", name=f"kT{jt}")
                ops = []
                for c4 in range(4):
                    ops.extend(proj_chunk_ops(wk, bkr, kT, c4, f"k{jt}"))
                for c4 in range(4):
                    ops.extend(proj_chunk_ops(wq, bqr, qT, c4, f"q{jt}"))
                return qT, kT, ops

            def outproj_qt_ops(qt):
                """One q-tile of y: accumulate all 4 pairs in PSUM, 1 output."""
                st = {}
                ops = []

                def mk(e, jh):
                    def op():
                        if jh == 0:
                            st[e] = fillps.tile([P, 512], f32, tag="fill",
                                                name=f"yp{qt}_{e}")
                        for j in (jh * 2, jh * 2 + 1):
                            nc.tensor.matmul(
                                st[e][:],
                                outh_t[j][:, qt * P:(qt + 1) * P],
                                wo_t[j][:, e * 512:(e + 1) * 512],
                                start=(j == 0), stop=(j == 3),
                            )
                    return op

                def fin():
                    yb = ysbpool.tile([P, 1024], f32, tag="ysb",
                                      name=f"ysb{qt}")
                    nc.vector.tensor_copy(yb[:, 0:512], st[0][:])
                    nc.vector.tensor_copy(yb[:, 512:1024], st[1][:])
                    nc.sync.dma_start(y_d[qt * P:(qt + 1) * P, :], yb[:])

                ops = [mk(0, 0), mk(0, 1), mk(1, 0), mk(1, 1), fin]
                return ops

            # ---- normalization stages (pipelined into the next window) ------
            def norm_stage1(jt, qc, pvA, pvB):
                """Evict pv (frees PSUM), gather denominators. DVE only."""
                pvca = pvcpool.tile([65, 512], f32, tag="pvc",
                                    name=f"pvc{jt}_{qc}_0")
                pvcb = pvcpool.tile([65, 512], f32, tag="pvc2",
                                    name=f"pvc{jt}_{qc}_1")
                nc.vector.tensor_copy(pvca[:], pvA[:])
                nc.vector.tensor_copy(pvcb[:], pvB[:])
                dnt = dnpool.tile([1, 1024], f32, tag="dn", name=f"dn{jt}_{qc}")
                nc.vector.tensor_copy(dnt[0:1, 0:512], pvca[64:65, :])
                nc.vector.tensor_copy(dnt[0:1, 512:1024], pvcb[64:65, :])
                return {"jt": jt, "qc": qc, "pvc": (pvca, pvcb), "dn": dnt}

            def norm_recip(ns):
                rcf = rcpool.tile([1, 1024], f32, tag="rcf",
                                  name=f"rcf{ns['jt']}_{ns['qc']}")
                nc.vector.reciprocal_approx_fast(out=rcf[:], in_=ns["dn"][:])
                ns["rc"] = rcf

            def norm_mul(ns, h2):
                jt, qc = ns["jt"], ns["qc"]
                bc = bcpool.tile([64, 512], f32, tag="bc",
                                 name=f"bc{jt}_{qc}_{h2}")
                nc.gpsimd.partition_broadcast(
                    bc[:], ns["rc"][0:1, h2 * 512:(h2 + 1) * 512])
                nc.vector.tensor_mul(
                    outh_t[jt][h2 * 64:h2 * 64 + 64, qc * 512:(qc + 1) * 512],
                    ns["pvc"][h2][0:64, :], bc[:])

            # ---- upfront: pair-0 weights, K + Q0 proj, full V c=0 ------------
            # DMA emission order = first-need order: xT chunk 0, pair-0
            # weights, wv (V proj), remaining xT chunks
            dma_xT_chunk(0)
            pw = {0: load_pair_weights(0)}
            nc.sync.dma_start(wv[:], wv_d[:].bitcast(f32r))
            for cc in range(1, 4):
                dma_xT_chunk(cc)
            qk = {}
            qT0 = qkpool.tile([P, S], f32r, tag="qT", name="qT0")
            kT0 = qkpool.tile([P, S], f32r, tag="kT", name="kT0")
            qk[0] = (qT0, kT0)
            for cc in range(4):
                for op in proj_chunk_ops(pw[0][1], pw[0][3], kT0, cc, "k0"):
                    op()
                if cc == 0:
                    for op in proj_chunk_ops(pw[0][0], pw[0][2], qT0, 0, "q0"):
                        op()
                for kt in range(cc * 4, min(cc * 4 + 4, 12)):
                    for op in v_chunk_ops(kt, 0):
                        op()

            # ---- attention: 4 pairs x 4 q-chunks x 16 k-tiles ----------------
            pending = None       # normalization state carried into next window
            for jt in range(NPAIR):
                qT, kT = qk[jt]

                # opportunistic filler queue for this pair
                fillers = []
                if jt == 0:
                    # last V c=0 chunks first (consumed by pv at kt 12-15)
                    for kt in range(12, 16):
                        fillers.extend(v_chunk_ops(kt, 0))
                    # remaining Q-proj chunks for pair 0 (needed at qc 1,2,3)
                    for c4 in range(1, 4):
                        fillers.extend(
                            proj_chunk_ops(pw[0][0], pw[0][2], qT0, c4, "q0"))
                    # second-half V columns, first 6 k-tiles
                    for kt in range(6):
                        fillers.extend(v_chunk_ops(kt, 1))
                if jt in (0, 1, 2):
                    pw[jt + 1] = load_pair_weights(jt + 1)
                    qTn, kTn, opsn = proj_pair_ops(jt + 1, *pw[jt + 1])
                    qk[jt + 1] = (qTn, kTn)
                    fillers.extend(opsn)
                if jt == 1:
                    for kt in range(6, 16):
                        fillers.extend(v_chunk_ops(kt, 1))
                fillers.reverse()  # pop() from the front, in order

                n_iters = 64
                it = 0
                for qc in range(4):
                    qs = slice(qc * 512, (qc + 1) * 512)
                    pvA = pvps.tile([65, 512], f32, tag="pvA",
                                    name=f"pvA{jt}_{qc}")
                    pvB = pvps.tile([65, 512], f32, tag="pvB",
                                    name=f"pvB{jt}_{qc}")
                    opq = []
                    if jt == 3 and qc >= 1:
                        for qt in range((qc - 1) * 4, (qc - 1) * 4 + 4):
                            opq.extend(outproj_qt_ops(qt))
                    prev_pv = None
                    for kt in range(16):
                        # scores for both heads (concurrent row-group matmuls)
                        sct = scps.tile([P, 1024], f32, tag="sc")
                        for h2 in range(2):
                            hb = h2 * 64
                            nc.tensor.matmul(
                                sct[:, h2 * 512:(h2 + 1) * 512],
                                kT[hb:hb + 64, kt * P:(kt + 1) * P],
                                qT[hb:hb + 64, qs],
                                start=True, stop=True,
                            )
                        et = etpool.tile([P, 1024], bf16, tag="exp")
                        nc.scalar.activation(
                            out=et[:], in_=sct[:], func=Exp,
                            bias=neg_c[:], scale=INV_SCALE,
                        )

                        # scheduled work for this slot (never blocks the
                        # scores/exp stream)
                        if kt == 2 and pending is not None:
                            norm_recip(pending)
                        elif kt == 4 and pending is not None:
                            norm_mul(pending, 0)
                        elif kt == 6 and pending is not None:
                            norm_mul(pending, 1)
                            pending = None
                        elif kt >= 7 and opq:
                            opq.pop(0)()
                            if opq:
                                opq.pop(0)()

                        # opportunistic fillers
                        it += 1
                        remaining = n_iters - it
                        budget = 2 if len(fillers) > remaining else (
                            1 if fillers else 0)
                        for _ in range(budget):
                            if fillers:
                                fillers.pop()()

                        # previous iteration's PV (after this iter's scores so
                        # it can't head-of-line-block them while waiting on exp)
                        if prev_pv is not None:
                            prev_pv()

                        def mk_pv(et_=et, kt_=kt):
                            def op():
                                for h2, pv in ((0, pvA), (1, pvB)):
                                    h = jt * 2 + h2
                                    nc.tensor.matmul(
                                        pv[:],
                                        v_sb[:, kt_, h * 65:h * 65 + 65],
                                        et_[:, h2 * 512:(h2 + 1) * 512],
                                        start=(kt_ == 0), stop=(kt_ == 15),
                                    )
                            return op
                        prev_pv = mk_pv()
                    prev_pv()
                    while opq:
                        opq.pop(0)()
                    while fillers and qc == 3 and jt < 3:
                        fillers.pop()()  # safety drain (should be empty)
                    pending_new = norm_stage1(jt, qc, pvA, pvB)
                    if pending is not None:
                        # should not happen (slots above consume it), but keep
                        # the chain correct if schedule shifts
                        norm_recip(pending)
                        norm_mul(pending, 0)
                        norm_mul(pending, 1)
                    pending = pending_new

            # ---- tail: last normalization + final out-projection -------------
            norm_recip(pending)
            norm_mul(pending, 0)
            norm_mul(pending, 1)
            for qt in range(12, 16):
                for op in outproj_qt_ops(qt):
                    op()

    nc.compile()
    return nc


def _get_nc():
    global _BUILT
    if _BUILT is None:
        _BUILT = _build()
    return _BUILT


def _itile_pack(m):
    """[E, C] -> [128, 8, C]: partition-major i-tile layout for 1-DMA loads."""
    return np.ascontiguousarray(
        m.reshape(8, P, m.shape[1]).transpose(1, 0, 2))


def _xt_pack(m):
    """[E, S] -> [128, 4, 8, 512]: per-partition-contiguous chunk-major."""
    return np.ascontiguousarray(
        m.reshape(8, P, 4, 512).transpose(1, 2, 0, 3))


def _w_pack(m):
    """[E, 512] -> [128, 4, 8, 128]: per-partition-contiguous pair-major."""
    return np.ascontiguousarray(
        m.reshape(8, P, 4, P).transpose(1, 2, 0, 3))


def _prep_core_inputs(x, Wq, bq, Wk, bk, Wv, bv, Wo, g, b):
    gs = g * 512
    xT = _xt_pack(x[b].T.astype(np.float32))
    wq = _w_pack(np.ascontiguousarray(Wq[:, gs:gs + 512].astype(np.float32)))
    wk = _w_pack(np.ascontiguousarray(Wk[:, gs:gs + 512].astype(np.float32)))
    bqs = np.ascontiguousarray(bq[gs:gs + 512].astype(np.float32).reshape(4, P, 1))
    bks = np.ascontiguousarray(bk[gs:gs + 512].astype(np.float32).reshape(4, P, 1))
    wv = np.zeros((E, 520), np.float32)
    bva = np.zeros((1, 1032), np.float32)
    bva[0, 520:] = 1.0
    for h in range(HCORE):
        wv[:, h * 65:h * 65 + 64] = Wv[:, gs + h * 64:gs + (h + 1) * 64]
        bva[0, h * 65:h * 65 + 64] = bv[gs + h * 64:gs + (h + 1) * 64]
        bva[0, h * 65 + 64] = 1.0
    wv = _itile_pack(wv)
    wo = np.ascontiguousarray(Wo[gs:gs + 512, :].astype('bfloat16'))
    return {
        "xT": xT, "wq": wq, "wk": wk, "bq": bqs, "bk": bks,
        "wv": wv, "bv": bva, "wo": wo,
    }


def kernel(x, Wq, bq, Wk, bk, Wv, bv, Wo, bo):
    from concourse.bass_utils import run_bass_kernel_spmd

    x = np.asarray(x)
    B = x.shape[0]
    nc = _get_nc()
    in_maps = []
    for c in range(8):
        g, b = c // 4, c % 4
        in_maps.append(
            _prep_core_inputs(x, np.asarray(Wq), np.asarray(bq), np.asarray(Wk),
                              np.asarray(bk), np.asarray(Wv), np.asarray(bv),
                              np.asarray(Wo), g, b)
        )
    res = run_bass_kernel_spmd(nc, in_maps, list(range(8)))
    y = np.zeros((B, S, E), np.float32)
    bo = np.asarray(bo, dtype=np.float32)
    for c in range(8):
        b = c % 4
        y[b] += res.results[c]["y"]
    y += bo
    return y


# revision 37
# speedup vs baseline: 1.3069x; 1.0066x over previous
"""Trainium2 Bass kernel for nn_MultiHeadAttention_60851096649901.

Sharding: 8 cores = 4 batches x 2 head-groups (8 heads each).
Each core computes its batch's attention for its 8 heads plus the full
out-projection partial for its head group; host sums the two head-group
partials and adds bo.

Per-core structure (v2 — pipelined emission, warm PE):
  qT/kT = (Wg.T @ x.T + b)           [128, 2048] f32r per head-pair
  v_aug = x @ Wv_aug + bv_aug        [2048, 520] bf16 (65 cols/head, 65th = 1)
  attention per pair, per q-chunk of 512, per k-tile of 128:
    scores for both heads via concurrent row-group matmuls -> [128, 1024] PSUM
    p = exp(8*s - 100) in one [128,1024] ACT instr -> et (f32r SBUF)
    pv[65, 512] += v_aug.T @ p  (PSUM accumulate over 16 k-tiles)
  emission is software-pipelined: scores(kt), exp(kt), fillers, pv(kt-1) --
  so the PV matmul (which waits on exp) never head-of-line-blocks the next
  scores matmul on the PE queue, and the exp stream runs back-to-back.
  normalization (per q-chunk, scheduled into the NEXT window so nothing
  stalls): evict pv, gather both heads' denominators into [2,512], one
  reciprocal_approx_fast, PE outer-product broadcast, DVE multiply.
  out-projection: per q-tile, 8 matmuls accumulate all 4 pairs into one
  PSUM tile (single y output, 8 MB not 32 MB); q-tiles 0-11 run as fillers
  during pair-3 attention, 12-15 in the tail.
PSUM banks: scores ring 2x[128,1024] (4) + pvA/pvB (2) + filler ring (2).
"""

import numpy as np

S = 2048
E = 1024
D = 64
P = 128
HCORE = 8          # heads per core
NPAIR = 4          # head-pairs per core
C_OFF = 100.0      # softmax constant offset (exp(8*s - C))
INV_SCALE = 8.0    # sqrt(head_dim)

_BUILT = None


def _build():
    import concourse.bass as bass
    import concourse.tile as tile
    from concourse import bacc, mybir

    f32 = mybir.dt.float32
    f32r = mybir.dt.float32r
    bf16 = mybir.dt.bfloat16
    Exp = mybir.ActivationFunctionType.Exp

    nc = bacc.Bacc("TRN2", target_bir_lowering=False, debug=False, num_devices=8)

    # host pre-packs weights/activations so every DMA is contiguous per
    # partition (descriptor-gen cost scales with segment count): xT is
    # token-chunk-major, wq/wk are pair-major
    xT_d = nc.dram_tensor("xT", [P, 4, 8, 512], f32, kind="ExternalInput")
    wq_d = nc.dram_tensor("wq", [P, 4, 8, P], f32, kind="ExternalInput")
    wk_d = nc.dram_tensor("wk", [P, 4, 8, P], f32, kind="ExternalInput")
    bq_d = nc.dram_tensor("bq", [4, P, 1], f32, kind="ExternalInput")
    bk_d = nc.dram_tensor("bk", [4, P, 1], f32, kind="ExternalInput")
    wv_d = nc.dram_tensor("wv", [P, 8, 520], f32, kind="ExternalInput")
    bv_d = nc.dram_tensor("bv", [1, 1032], f32, kind="ExternalInput")
    wo_d = nc.dram_tensor("wo", [512, E], bf16, kind="ExternalInput")
    y_d = nc.dram_tensor("y", [S, E], f32, kind="ExternalOutput")

    with tile.TileContext(nc) as tc:
        with (
            tc.tile_pool(name="persist", bufs=1) as persist,
            tc.tile_pool(name="wpool", bufs=2) as wpool,
            tc.tile_pool(name="qk", bufs=2) as qkpool,
            tc.tile_pool(name="et", bufs=3) as etpool,
            tc.tile_pool(name="pvc", bufs=2) as pvcpool,
            tc.tile_pool(name="dn", bufs=1) as dnpool,
            tc.tile_pool(name="rcp", bufs=1) as rcpool,
            tc.tile_pool(name="bcp", bufs=2) as bcpool,
            tc.tile_pool(name="ysb", bufs=2) as ysbpool,
            tc.tile_pool(name="sc", bufs=2, space="PSUM") as scps,     # scores only
            tc.tile_pool(name="pv", bufs=1, space="PSUM") as pvps,     # pv A/B
            tc.tile_pool(name="fill", bufs=2, space="PSUM") as fillps,  # everything else
        ):
            # ---- persistent tiles -------------------------------------------
            # xT layout: [i-part, token-chunk, i-tile, token-within-chunk]
            xT = persist.tile([P, 4, 8, 512], f32r, tag="xT")

            def xt_proj(i, c4):
                """rhs slice for projections: i-tile x 512-token chunk."""
                return xT[:, c4, i, :]

            def xt_ktile(i, kt):
                """lhsT slice for the V projection: i-tile x 128-token tile."""
                o = (kt % 4) * P
                return xT[:, kt // 4, i, o:o + P]
            v_sb = persist.tile([P, 16, 520], bf16, tag="v_sb")
            wv = persist.tile([P, 8, 520], f32r, tag="wv")

            neg_c = persist.tile([P, 1], f32, tag="neg_c")
            nc.vector.memset(neg_c[:], -C_OFF)

            bv_r = persist.tile([1, 1032], f32r, tag="bv_r")
            nc.sync.dma_start(bv_r[:], bv_d[:].bitcast(f32r))
            ones_r = bv_r[:, 520:1032]  # host packs ones after bv_aug

            # all 4 pairs' Wo slices and outhT persist until the out-projection
            wo_t = [persist.tile([P, E], bf16, tag=f"wo{j}", name=f"wo{j}")
                    for j in range(NPAIR)]
            outh_t = [persist.tile([P, S], bf16, tag=f"oh{j}", name=f"oh{j}")
                      for j in range(NPAIR)]

            def dma_xT_chunk(cc):
                nc.sync.dma_start(xT[:, cc], xT_d[:, cc].bitcast(f32r))

            # ---- op generators (each closure ~1-2 matmuls of PE work) -------
            def v_chunk_ops(kt, c):
                """v_aug[:, kt, chunk c] = x @ Wv_aug + bv (5 closures)."""
                st = {}
                cs = slice(c * 260, (c + 1) * 260)
                ops = []

                def mk(i0):
                    def op():
                        if i0 == 0:
                            st["p"] = fillps.tile([P, 260], f32, tag="fill",
                                                  name=f"vps{kt}_{c}")
                        for i in (i0, i0 + 1):
                            nc.tensor.matmul(
                                st["p"][:], xt_ktile(i, kt),
                                wv[:, i, cs], start=(i == 0), stop=False,
                            )
                    return op

                for i0 in range(0, 8, 2):
                    ops.append(mk(i0))

                def fin():
                    nc.tensor.matmul(
                        st["p"][:], ones_r[:, 0:P], bv_r[:, cs],
                        start=False, stop=True,
                    )
                    nc.vector.tensor_copy(v_sb[:, kt, cs], st["p"][:])
                ops.append(fin)
                return ops

            def proj_chunk_ops(w, br, dst, c4, nm):
                """qT/kT 512-token chunk c4: 5 closures (8 MM + bias + evict)."""
                st = {}
                ops = []

                def mk(i0):
                    def op():
                        if i0 == 0:
                            st["p"] = fillps.tile([P, 512], f32, tag="fill",
                                                  name=f"pp{nm}_{c4}")
                        for i in (i0, i0 + 1):
                            nc.tensor.matmul(
                                st["p"][:], w[:, i, :], xt_proj(i, c4),
                                start=(i == 0), stop=False,
                            )
                    return op

                for i0 in range(0, 8, 2):
                    ops.append(mk(i0))

                def fin():
                    # bias is per-partition (per output dim) -> fold into the
                    # DVE evict instead of a PE matmul
                    nc.vector.tensor_scalar_add(
                        dst[:, c4 * 512:(c4 + 1) * 512], st["p"][:], br[:])
                ops.append(fin)
                return ops

            def load_pair_weights(jt):
                js = slice(jt * P, (jt + 1) * P)
                wq = wpool.tile([P, 8, P], f32r, tag="wq", name=f"wq{jt}")
                wk = wpool.tile([P, 8, P], f32r, tag="wk", name=f"wk{jt}")
                nc.sync.dma_start(wq[:], wq_d[:, jt].bitcast(f32r))
                nc.sync.dma_start(wk[:], wk_d[:, jt].bitcast(f32r))
                bqr = wpool.tile([P, 1], f32, tag="bqr", name=f"bqr{jt}")
                bkr = wpool.tile([P, 1], f32, tag="bkr", name=f"bkr{jt}")
                nc.sync.dma_start(bqr[:], bq_d[jt])
                nc.sync.dma_start(bkr[:], bk_d[jt])
                nc.sync.dma_start(wo_t[jt][:], wo_d[js, :])
                return wq, wk, bqr, bkr

            def proj_pair_ops(jt, wq, wk, bqr, bkr):
                """Interleave K/Q chunks in consumption order (K first)."""
                qT = qkpool.tile([P, S], f32r, tag="qT", name=f"qT{jt}")
                kT = qkpool.tile([P, S], f32r, tag="kT", name=f"kT{jt}")
                ops = []
                for c4 in range(4):
                    ops.extend(proj_chunk_ops(wk, bkr, kT, c4, f"k{jt}"))
                for c4 in range(4):
                    ops.extend(proj_chunk_ops(wq, bqr, qT, c4, f"q{jt}"))
                return qT, kT, ops

            def outproj_qt_ops(qt):
                """One q-tile of y: accumulate all 4 pairs in PSUM, 1 output."""
                st = {}
                ops = []

                def mk(e, jh):
                    def op():
                        if jh == 0:
                            st[e] = fillps.tile([P, 512], f32, tag="fill",
                                                name=f"yp{qt}_{e}")
                        for j in (jh * 2, jh * 2 + 1):
                            nc.tensor.matmul(
                                st[e][:],
                                outh_t[j][:, qt * P:(qt + 1) * P],
                                wo_t[j][:, e * 512:(e + 1) * 512],
                                start=(j == 0), stop=(j == 3),
                            )
                    return op

                def fin():
                    yb = ysbpool.tile([P, 1024], f32, tag="ysb",
                                      name=f"ysb{qt}")
                    nc.vector.tensor_copy(yb[:, 0:512], st[0][:])
                    nc.vector.tensor_copy(yb[:, 512:1024], st[1][:])
                    nc.sync.dma_start(y_d[qt * P:(qt + 1) * P, :], yb[:])

                ops = [mk(0, 0), mk(0, 1), mk(1, 0), mk(1, 1), fin]
                return ops

            # ---- normalization stages (pipelined into the next window) ------
            def norm_stage1(jt, qc, pvA, pvB):
                """Evict pv (frees PSUM), gather denominators. DVE only."""
                pvca = pvcpool.tile([65, 512], f32, tag="pvc",
                                    name=f"pvc{jt}_{qc}_0")
                pvcb = pvcpool.tile([65, 512], f32, tag="pvc2",
                                    name=f"pvc{jt}_{qc}_1")
                nc.vector.tensor_copy(pvca[:], pvA[:])
                nc.vector.tensor_copy(pvcb[:], pvB[:])
                dnt = dnpool.tile([1, 1024], f32, tag="dn", name=f"dn{jt}_{qc}")
                nc.vector.tensor_copy(dnt[0:1, 0:512], pvca[64:65, :])
                nc.vector.tensor_copy(dnt[0:1, 512:1024], pvcb[64:65, :])
                return {"jt": jt, "qc": qc, "pvc": (pvca, pvcb), "dn": dnt}

            def norm_recip(ns):
                rcf = rcpool.tile([1, 1024], f32, tag="rcf",
                                  name=f"rcf{ns['jt']}_{ns['qc']}")
                nc.vector.reciprocal_approx_fast(out=rcf[:], in_=ns["dn"][:])
                ns["rc"] = rcf

            def norm_mul(ns, h2):
                jt, qc = ns["jt"], ns["qc"]
                bc = bcpool.tile([64, 512], f32, tag="bc",
                                 name=f"bc{jt}_{qc}_{h2}")
                nc.gpsimd.partition_broadcast(
                    bc[:], ns["rc"][0:1, h2 * 512:(h2 + 1) * 512])
                nc.vector.tensor_mul(
                    outh_t[jt][h2 * 64:h2 * 64 + 64, qc * 512:(qc + 1) * 512],
                    ns["pvc"][h2][0:64, :], bc[:])

            # ---- upfront: pair-0 weights, K + Q0 proj, full V c=0 ------------
            # DMA emission order = first-need order: xT chunk 0, pair-0
            # weights, wv (V proj), remaining xT chunks
            dma_xT_chunk(0)
            pw = {0: load_pair_weights(0)}
            nc.sync.dma_start(wv[:], wv_d[:].bitcast(f32r))
            for cc in range(1, 4):
                dma_xT_chunk(cc)
            qk = {}
            qT0 = qkpool.tile([P, S], f32r, tag="qT", name="qT0")
            kT0 = qkpool.tile([P, S], f32r, tag="kT", name="kT0")
            qk[0] = (qT0, kT0)
            for cc in range(4):
                for op in proj_chunk_ops(pw[0][1], pw[0][3], kT0, cc, "k0"):
                    op()
                if cc == 0:
                    for op in proj_chunk_ops(pw[0][0], pw[0][2], qT0, 0, "q0"):
                        op()
                for kt in range(cc * 4, min(cc * 4 + 4, 12)):
                    for op in v_chunk_ops(kt, 0):
                        op()

            # ---- attention: 4 pairs x 4 q-chunks x 16 k-tiles ----------------
            pending = None       # normalization state carried into next window
            carry = None         # closure: previous window's last pv + evict
            for jt in range(NPAIR):
                qT, kT = qk[jt]

                # opportunistic filler queue for this pair
                fillers = []
                if jt == 0:
                    # last V c=0 chunks first (consumed by pv at kt 12-15)
                    for kt in range(12, 16):
                        fillers.extend(v_chunk_ops(kt, 0))
                    # remaining Q-proj chunks for pair 0 (needed at qc 1,2,3)
                    for c4 in range(1, 4):
                        fillers.extend(
                            proj_chunk_ops(pw[0][0], pw[0][2], qT0, c4, "q0"))
                    # second-half V columns, first 6 k-tiles
                    for kt in range(6):
                        fillers.extend(v_chunk_ops(kt, 1))
                if jt in (0, 1, 2):
                    pw[jt + 1] = load_pair_weights(jt + 1)
                    qTn, kTn, opsn = proj_pair_ops(jt + 1, *pw[jt + 1])
                    qk[jt + 1] = (qTn, kTn)
                    fillers.extend(opsn)
                if jt == 1:
                    for kt in range(6, 16):
                        fillers.extend(v_chunk_ops(kt, 1))
                fillers.reverse()  # pop() from the front, in order

                n_iters = 64
                it = 0
                for qc in range(4):
                    qs = slice(qc * 512, (qc + 1) * 512)
                    pvA = pvps.tile([65, 512], f32, tag="pvA",
                                    name=f"pvA{jt}_{qc}")
                    pvB = pvps.tile([65, 512], f32, tag="pvB",
                                    name=f"pvB{jt}_{qc}")
                    opq = []
                    if jt == 3 and qc >= 1:
                        for qt in range((qc - 1) * 4, (qc - 1) * 4 + 4):
                            opq.extend(outproj_qt_ops(qt))
                    prev_pv = None
                    for kt in range(16):
                        # scores for both heads (concurrent row-group matmuls)
                        sct = scps.tile([P, 1024], f32, tag="sc")
                        for h2 in range(2):
                            hb = h2 * 64
                            nc.tensor.matmul(
                                sct[:, h2 * 512:(h2 + 1) * 512],
                                kT[hb:hb + 64, kt * P:(kt + 1) * P],
                                qT[hb:hb + 64, qs],
                                start=True, stop=True,
                            )
                        et = etpool.tile([P, 1024], bf16, tag="exp")
                        nc.scalar.activation(
                            out=et[:], in_=sct[:], func=Exp,
                            bias=neg_c[:], scale=INV_SCALE,
                        )

                        # scheduled work for this slot (never blocks the
                        # scores/exp stream)
                        if kt == 0 and carry is not None:
                            carry()
                            carry = None
                        elif kt == 2 and pending is not None:
                            norm_recip(pending)
                        elif kt == 4 and pending is not None:
                            norm_mul(pending, 0)
                        elif kt == 6 and pending is not None:
                            norm_mul(pending, 1)
                            pending = None
                        elif opq and kt >= 7:
                            opq.pop(0)()
                            if opq:
                                opq.pop(0)()

                        # opportunistic fillers
                        it += 1
                        remaining = n_iters - it
                        budget = 2 if len(fillers) > remaining else (
                            1 if fillers else 0)
                        for _ in range(budget):
                            if fillers:
                                fillers.pop()()

                        # previous iteration's PV (after this iter's scores so
                        # it can't head-of-line-block them while waiting on exp)
                        if prev_pv is not None:
                            prev_pv()

                        def mk_pv(et_=et, kt_=kt, pvA_=pvA, pvB_=pvB, jt_=jt):
                            def op():
                                for h2, pv in ((0, pvA_), (1, pvB_)):
                                    h = jt_ * 2 + h2
                                    nc.tensor.matmul(
                                        pv[:],
                                        v_sb[:, kt_, h * 65:h * 65 + 65],
                                        et_[:, h2 * 512:(h2 + 1) * 512],
                                        start=(kt_ == 0), stop=(kt_ == 15),
                                    )
                            return op
                        prev_pv = mk_pv()
                    while opq:
                        opq.pop(0)()
                    while fillers and qc == 3 and jt < 3:
                        fillers.pop()()  # safety drain (should be empty)

                    # defer this window's last pv + eviction into the next
                    # window (emitted after its first scores/exp, so the qc
                    # boundary never head-of-line-blocks the PE queue)
                    def mk_carry(jt_=jt, qc_=qc, pvA_=pvA, pvB_=pvB,
                                 last_pv=prev_pv):
                        def c():
                            nonlocal pending
                            last_pv()
                            pending = norm_stage1(jt_, qc_, pvA_, pvB_)
                        return c
                    carry = mk_carry()

            # ---- tail: last pv + normalization + final out-projection --------
            carry()
            carry = None
            norm_recip(pending)
            norm_mul(pending, 0)
            norm_mul(pending, 1)
            for qt in range(12, 16):
                for op in outproj_qt_ops(qt):
                    op()

    nc.compile()
    return nc


def _get_nc():
    global _BUILT
    if _BUILT is None:
        _BUILT = _build()
    return _BUILT


def _itile_pack(m):
    """[E, C] -> [128, 8, C]: partition-major i-tile layout for 1-DMA loads."""
    return np.ascontiguousarray(
        m.reshape(8, P, m.shape[1]).transpose(1, 0, 2))


def _xt_pack(m):
    """[E, S] -> [128, 4, 8, 512]: per-partition-contiguous chunk-major."""
    return np.ascontiguousarray(
        m.reshape(8, P, 4, 512).transpose(1, 2, 0, 3))


def _w_pack(m):
    """[E, 512] -> [128, 4, 8, 128]: per-partition-contiguous pair-major."""
    return np.ascontiguousarray(
        m.reshape(8, P, 4, P).transpose(1, 2, 0, 3))


def _prep_core_inputs(x, Wq, bq, Wk, bk, Wv, bv, Wo, g, b):
    gs = g * 512
    xT = _xt_pack(x[b].T.astype(np.float32))
    wq = _w_pack(np.ascontiguousarray(Wq[:, gs:gs + 512].astype(np.float32)))
    wk = _w_pack(np.ascontiguousarray(Wk[:, gs:gs + 512].astype(np.float32)))
    bqs = np.ascontiguousarray(bq[gs:gs + 512].astype(np.float32).reshape(4, P, 1))
    bks = np.ascontiguousarray(bk[gs:gs + 512].astype(np.float32).reshape(4, P, 1))
    wv = np.zeros((E, 520), np.float32)
    bva = np.zeros((1, 1032), np.float32)
    bva[0, 520:] = 1.0
    for h in range(HCORE):
        wv[:, h * 65:h * 65 + 64] = Wv[:, gs + h * 64:gs + (h + 1) * 64]
        bva[0, h * 65:h * 65 + 64] = bv[gs + h * 64:gs + (h + 1) * 64]
        bva[0, h * 65 + 64] = 1.0
    wv = _itile_pack(wv)
    wo = np.ascontiguousarray(Wo[gs:gs + 512, :].astype('bfloat16'))
    return {
        "xT": xT, "wq": wq, "wk": wk, "bq": bqs, "bk": bks,
        "wv": wv, "bv": bva, "wo": wo,
    }


def kernel(x, Wq, bq, Wk, bk, Wv, bv, Wo, bo):
    from concourse.bass_utils import run_bass_kernel_spmd

    x = np.asarray(x)
    B = x.shape[0]
    nc = _get_nc()
    in_maps = []
    for c in range(8):
        g, b = c // 4, c % 4
        in_maps.append(
            _prep_core_inputs(x, np.asarray(Wq), np.asarray(bq), np.asarray(Wk),
                              np.asarray(bk), np.asarray(Wv), np.asarray(bv),
                              np.asarray(Wo), g, b)
        )
    res = run_bass_kernel_spmd(nc, in_maps, list(range(8)))
    y = np.zeros((B, S, E), np.float32)
    bo = np.asarray(bo, dtype=np.float32)
    for c in range(8):
        b = c % 4
        y[b] += res.results[c]["y"]
    y += bo
    return y
